# revision 26
# baseline (speedup 1.0000x reference)
"""Trainium2 Bass kernel for nn_BestHits: out = bh * bh.T where
bh = blockwise-softmax(mask_diag(similarities) / TAU) over 256-wide column groups.

Strategy: out is symmetric, so only the 136 upper-triangle 512x512 block-pairs
are computed (17 per core on 8 cores); the host mirrors out[J,I] = out[I,J].T.

Per pair (I, J) the core computes out[I,J] = bhA * bhB.T with A = sims[I,J],
B = sims[J,I]. The HBM-traffic and engine plan (target_regime=memory):

  - Inputs are staged fp16 and the B block is staged TRANSPOSED on the host,
    so each slot is ONE 1-MiB load [a | bT] and the kernel never runs
    transpose matmuls. Outputs are stored bf16 (host converts to fp32).
    Rel-err budget: ~5e-3 measured vs the 2e-2 gate.
  - ACT: two big exps za = exp(a/TAU - C), zbt = exp(bT/TAU - C) -> bf16.
    The -C bias cancels in the softmax and keeps every intermediate (z,
    sums, 1/sa * 1/sb products) inside fp32/bf16 range.
  - DVE: one free-axis group-sum reduce for A; B's group sums are over bT's
    PARTITION axis, which the PE computes with ones-matmuls (free-axis
    reduction is DVE-only and DVE is the bottleneck engine).
  - PE also builds the combined normalizer psum_norm[p, f] =
    ra[p, g(f)] * rb[f, g(p)] as 16 rank-1 (K=1) outer-product matmuls from
    the transposed reciprocal rows (one 128x16 PE transpose per slot).
  - DVE finishes with two big multiplies: tmp = za*zbt (bf16, 2x mode) and
    out = tmp * psum_norm (1x, PSUM operand), 8 ops/slot total instead of
    the baseline's ~17.

Per-core HBM traffic: 17 MiB loads + 8.5 MiB stores (vs 49 MiB fp32).
"""
import sys

import numpy as np

sys.path.insert(0, "/opt/trn_rl_repo")

from contextlib import ExitStack

import concourse.bass as bass  # noqa: F401  (registers AP machinery)
import concourse.tile as tile
from concourse import bacc, masks, mybir
from concourse.bass_utils import run_bass_kernel_spmd

N = 8192          # full matrix side
BLK = 512         # block side
NB = N // BLK     # 16 blocks per side
P = 128           # SBUF partitions
T = BLK // P      # 4 subtiles per block side
GRP = 256         # softmax group width
NG = BLK // GRP   # 2 groups per block side
TAU = 0.1
NDIAG = 2         # diagonal pairs per core (last slots; no special kernel path)
NSLOTS = 17       # block-pairs per core
NCORES = 8
MASK = -60000.0   # fp16-representable diag mask (exp(MASK/TAU - C) == 0)
C_BIAS = 28.0     # exp(x/TAU - C): cancels in softmax, prevents overflow

F16 = mybir.dt.float16
BF16 = mybir.dt.bfloat16
F32 = mybir.dt.float32
AF = mybir.ActivationFunctionType
OP = mybir.AluOpType

NX = T * NG       # 8 (t, g) group-sum columns per block


def core_pairs() -> list[list[tuple[int, int]]]:
    """136 upper-triangle block pairs distributed 17-per-core."""
    diag = [(i, i) for i in range(NB)]
    off = [(i, j) for i in range(NB) for j in range(i + 1, NB)]
    cps: list[list[tuple[int, int]]] = [[] for _ in range(NCORES)]
    for idx, pr in enumerate(off):
        cps[idx % NCORES].append(pr)
    for idx, pr in enumerate(diag):
        cps[idx % NCORES].append(pr)
    return cps


CORE_PAIRS = core_pairs()


def build():
    """Build + compile the (single-program, 8-core SPMD) Bass kernel."""
    nc = bacc.Bacc(
        "TRN2",
        target_bir_lowering=False,
        debug=False,
        enable_asserts=True,
        num_devices=NCORES,
    )
    x = nc.dram_tensor("x", [NSLOTS, P, 2, T, BLK], F16, kind="ExternalInput").ap()
    o = nc.dram_tensor("o", [NSLOTS, P, T, BLK], BF16, kind="ExternalOutput").ap()

    with tile.TileContext(nc) as tc, ExitStack() as ctx:
        const_pool = ctx.enter_context(tc.tile_pool(name="const", bufs=1))
        ident = const_pool.tile([P, P], F32)
        masks.make_identity(nc, ident[:])
        ones = const_pool.tile([P, 1], BF16)
        nc.gpsimd.memset(ones[:], 1.0)
        cbias = const_pool.tile([P, 1], F32)
        nc.gpsimd.memset(cbias[:], -C_BIAS)

        x_pool = ctx.enter_context(tc.tile_pool(name="x_sb", bufs=4))
        z_pool = ctx.enter_context(tc.tile_pool(name="z_sb", bufs=4))
        tmp_pool = ctx.enter_context(tc.tile_pool(name="tmp", bufs=3))
        o_pool = ctx.enter_context(tc.tile_pool(name="o_sb", bufs=3))
        s_pool = ctx.enter_context(tc.tile_pool(name="small", bufs=6))
        t_pool = ctx.enter_context(tc.tile_pool(name="tsb", bufs=4))
        ps_sb_pool = ctx.enter_context(tc.tile_pool(name="ps_sb", bufs=2, space="PSUM"))
        ps_t_pool = ctx.enter_context(tc.tile_pool(name="ps_t", bufs=2, space="PSUM"))
        ps_n_pool = ctx.enter_context(tc.tile_pool(name="ps_n", bufs=1, space="PSUM"))

        for k in range(NSLOTS):
            diag = k >= NSLOTS - NDIAG
            x_sb = x_pool.tile([P, 2, T, BLK], F16)
            nc.sync.dma_start(x_sb[:, 0], x[k][:, 0])
            nc.sync.dma_start(x_sb[:, 1], x[k][:, 1])

            za = z_pool.tile([P, T, BLK], BF16, name="za")
            zbt = z_pool.tile([P, T, BLK], BF16, name="zbt")
            nc.scalar.activation(za[:], x_sb[:, 0], AF.Exp, scale=1.0 / TAU,
                                 bias=cbias[:])
            nc.scalar.activation(zbt[:], x_sb[:, 1], AF.Exp, scale=1.0 / TAU,
                                 bias=cbias[:])

            # A-side group sums on DVE: sa[p, t*NG+g] over 256-wide col groups.
            # Diagonal slots skip this: B == A.T there, so the PE-computed
            # ps_sb below already holds A's row-group sums (columns permuted).
            if not diag:
                sa = s_pool.tile([P, NX], F32, name="sa")
                nc.vector.tensor_reduce(
                    sa[:],
                    za[:].rearrange("p t b -> p (t b)").rearrange(
                        "p (x s) -> p x s", s=GRP
                    ),
                    axis=mybir.AxisListType.X,
                    op=OP.add,
                )

            # B-side group sums on PE: zbt[p, t, y] = exp(B[y, t*128+p]) so a
            # ones-matmul contracts B-columns; accumulating subtile pairs
            # gives ps_sb[p, g*T+rc] = sum over group g for row rc*128+p.
            ps_sb = ps_sb_pool.tile([P, NX], F32)
            for g in range(NG):
                for rc in range(T):
                    for i in range(2):
                        nc.tensor.matmul(
                            ps_sb[:, g * T + rc:g * T + rc + 1],
                            zbt[:, 2 * g + i, rc * P:(rc + 1) * P],
                            ones[:],
                            start=(i == 0),
                            stop=(i == 1),
                        )

            # Reciprocals packed [ra | rb] (natural column order: ra x=t*NG+g,
            # rb x=8+g*T+rc), PE-transposed in fp32, cast to bf16 on the way
            # out of PSUM, then a single tiny SBUF->SBUF DMA flattens all 16
            # rows onto partition 0 so every matmul operand slice below is
            # base-partition-0 legal.
            r32 = s_pool.tile([P, 2, NX], F32, name="r32")
            if diag:
                nc.vector.reciprocal(
                    r32[:, 0].rearrange("p (t g) -> p t g", g=NG),
                    ps_sb[:].rearrange("p (g t) -> p t g", g=NG),
                )
            else:
                nc.vector.reciprocal(r32[:, 0], sa[:])
            nc.vector.reciprocal(r32[:, 1], ps_sb[:])
            ps_t = ps_t_pool.tile([2 * NX, P], F32)
            nc.tensor.transpose(
                ps_t[:], r32[:].rearrange("p a b -> p (a b)"), ident[:]
            )
            t_sb = t_pool.tile([2 * NX, P], BF16)
            nc.vector.tensor_scalar_mul(t_sb[:], ps_t[:], 1.0)
            rt = t_pool.tile([1, 2 * NX, P], BF16, name="rt")
            nc.gpsimd.dma_start(rt[:], t_sb[:])

            # tmp = za * zbt in bf16: row-subtiles 0-2 on the (otherwise idle)
            # gpsimd engine, subtile 3 on DVE in 2x mode.
            tmp = tmp_pool.tile([P, T, BLK], BF16)
            nc.gpsimd.tensor_mul(tmp[:, 0:3], za[:, 0:3], zbt[:, 0:3])
            nc.vector.tensor_mul(tmp[:, 3:4], za[:, 3:4], zbt[:, 3:4])

            # psum_norm[v][p, f] = ra[p, (v, f//256)] * rb[f, v//2] as K=1
            # outer-product matmuls (one per (v, f-half)); then
            # out[v] = tmp[v] * psum_norm[v] per DVE op.
            o_sb = o_pool.tile([P, T, BLK], BF16)
            for hv in range(2):
                ps_n = ps_n_pool.tile([P, 2, BLK], F32, name=f"psn{hv}")
                for dv in range(2):
                    v = 2 * hv + dv
                    g = v // 2
                    for h in range(2):
                        nc.tensor.matmul(
                            ps_n[:, dv, h * GRP:(h + 1) * GRP],
                            rt[0:1, v * NG + h, :],
                            rt[0:1, NX + g * T + 2 * h:NX + g * T + 2 * h + 2, :]
                            .rearrange("o a p -> o (a p)"),
                        )
                if hv == 0:
                    # ACT (idle after its exps) drains this half of PSUM to
                    # SBUF bf16 so the combine below runs in DVE 2x mode.
                    n_sb = s_pool.tile([P, 2, BLK], BF16, name="n_sb")
                    nc.scalar.copy(n_sb[:], ps_n[:])
                    nc.vector.tensor_mul(
                        o_sb[:, 0:2, :], tmp[:, 0:2, :], n_sb[:]
                    )
                else:
                    nc.vector.tensor_mul(
                        o_sb[:, 2:4, :], tmp[:, 2:4, :], ps_n[:]
                    )

            nc.gpsimd.dma_start(o[k], o_sb[:])

    nc.compile()
    return nc


_NC = None


def _get_nc():
    global _NC
    if _NC is None:
        _NC = build()
    return _NC


def _to_pmajor(block: np.ndarray) -> np.ndarray:
    # (512, 512) row-major -> (128, 4, 512): row r = t*P + p lands at [p, t, :]
    return block.reshape(T, P, BLK).transpose(1, 0, 2)


def make_in_maps(sims: np.ndarray) -> list[dict[str, np.ndarray]]:
    s16 = sims.astype(np.float16)
    in_maps = []
    for c in range(NCORES):
        xs = np.empty((NSLOTS, P, 2, T, BLK), np.float16)
        for k, (i, j) in enumerate(CORE_PAIRS[c]):
            a = s16[i * BLK:(i + 1) * BLK, j * BLK:(j + 1) * BLK]
            bt = s16[j * BLK:(j + 1) * BLK, i * BLK:(i + 1) * BLK].T
            if i == j:
                a = a.copy()
                np.fill_diagonal(a, MASK)
                bt = a.T
            xs[k, :, 0] = _to_pmajor(np.ascontiguousarray(a))
            xs[k, :, 1] = _to_pmajor(np.ascontiguousarray(bt))
        in_maps.append({"x": xs})
    return in_maps


def assemble(results: list[dict[str, np.ndarray]]) -> np.ndarray:
    out = np.empty((N, N), np.float32)
    for c in range(NCORES):
        o_pm = np.asarray(results[c]["o"]).astype(np.float32)
        o_stack = o_pm.transpose(0, 2, 1, 3).reshape(NSLOTS, BLK, BLK)
        for k, (i, j) in enumerate(CORE_PAIRS[c]):
            out[i * BLK:(i + 1) * BLK, j * BLK:(j + 1) * BLK] = o_stack[k]
            if i != j:
                out[j * BLK:(j + 1) * BLK, i * BLK:(i + 1) * BLK] = o_stack[k].T
    return out


def run_on_hw(sims: np.ndarray, **spmd_kwargs):
    """Run the kernel on the 8 NeuronCores. Returns (out, BassKernelResults).

    The device occasionally throws a transient NRT_EXEC_UNIT_UNRECOVERABLE
    and needs ~a minute to come back, so failed runs are retried."""
    import time

    nc = _get_nc()
    in_maps = make_in_maps(sims)
    last_exc = None
    for attempt in range(3):
        if attempt:
            time.sleep(75)
        try:
            res = run_bass_kernel_spmd(
                nc, in_maps, core_ids=list(range(NCORES)), **spmd_kwargs
            )
            return assemble(res.results), res
        except Exception as exc:  # noqa: BLE001 - device flake, retry
            last_exc = exc
    raise last_exc


def kernel(similarities: np.ndarray) -> np.ndarray:
    sims = np.ascontiguousarray(similarities, dtype=np.float32)
    assert sims.shape == (N, N)
    out, _ = run_on_hw(sims)
    return out


if __name__ == "__main__":
    rng = np.random.default_rng(0)
    sims = rng.standard_normal((N, N), dtype=np.float32)
    out = kernel(similarities=sims)
    print("out", out.shape, out.dtype, float(out.max()))


# revision 27
# speedup vs baseline: 1.1248x; 1.1248x over previous
"""Trainium2 Bass kernel for nn_BestHits: out = bh * bh.T where
bh = blockwise-softmax(mask_diag(similarities) / TAU) over 256-wide column groups.

Strategy: out is symmetric, so only the 136 upper-triangle 512x512 block-pairs
are computed (17 per core on 8 cores); the host mirrors out[J,I] = out[I,J].T.

Per pair (I, J) the core computes out[I,J] = bhA * bhB.T with A = sims[I,J],
B = sims[J,I]. The HBM-traffic and engine plan (target_regime=memory):

  - Inputs are staged fp16 and the B block is staged TRANSPOSED on the host,
    so each slot is ONE 1-MiB load [a | bT] and the kernel never runs
    transpose matmuls. Outputs are stored bf16 (host converts to fp32).
    Rel-err budget: ~5e-3 measured vs the 2e-2 gate.
  - ACT: two big exps za = exp(a/TAU - C), zbt = exp(bT/TAU - C) -> bf16.
    The -C bias cancels in the softmax and keeps every intermediate (z,
    sums, 1/sa * 1/sb products) inside fp32/bf16 range.
  - DVE: one free-axis group-sum reduce for A; B's group sums are over bT's
    PARTITION axis, which the PE computes with ones-matmuls (free-axis
    reduction is DVE-only and DVE is the bottleneck engine).
  - PE also builds the combined normalizer psum_norm[p, f] =
    ra[p, g(f)] * rb[f, g(p)] as 16 rank-1 (K=1) outer-product matmuls from
    the transposed reciprocal rows (one 128x16 PE transpose per slot).
  - DVE finishes with two big multiplies: tmp = za*zbt (bf16, 2x mode) and
    out = tmp * psum_norm (1x, PSUM operand), 8 ops/slot total instead of
    the baseline's ~17.

Per-core HBM traffic: 17 MiB loads + 8.5 MiB stores (vs 49 MiB fp32).
"""
import sys

import numpy as np

sys.path.insert(0, "/opt/trn_rl_repo")

from contextlib import ExitStack

import concourse.bass as bass  # noqa: F401  (registers AP machinery)
import concourse.tile as tile
from concourse import bacc, masks, mybir
from concourse.bass_utils import run_bass_kernel_spmd

N = 8192          # full matrix side
BLK = 512         # block side
NB = N // BLK     # 16 blocks per side
P = 128           # SBUF partitions
T = BLK // P      # 4 subtiles per block side
GRP = 256         # softmax group width
NG = BLK // GRP   # 2 groups per block side
TAU = 0.1
NDIAG = 2         # diagonal pairs per core (last slots; no special kernel path)
NSLOTS = 17       # block-pairs per core
NCORES = 8
MASK = -60000.0   # fp16-representable diag mask (exp(MASK/TAU - C) == 0)
C_BIAS = 28.0     # exp(x/TAU - C): cancels in softmax, prevents overflow

F16 = mybir.dt.float16
BF16 = mybir.dt.bfloat16
F32 = mybir.dt.float32
AF = mybir.ActivationFunctionType
OP = mybir.AluOpType

NX = T * NG       # 8 (t, g) group-sum columns per block


def core_pairs() -> list[list[tuple[int, int]]]:
    """136 upper-triangle block pairs distributed 17-per-core."""
    diag = [(i, i) for i in range(NB)]
    off = [(i, j) for i in range(NB) for j in range(i + 1, NB)]
    cps: list[list[tuple[int, int]]] = [[] for _ in range(NCORES)]
    for idx, pr in enumerate(off):
        cps[idx % NCORES].append(pr)
    for idx, pr in enumerate(diag):
        cps[idx % NCORES].append(pr)
    return cps


CORE_PAIRS = core_pairs()


def build():
    """Build + compile the (single-program, 8-core SPMD) Bass kernel."""
    nc = bacc.Bacc(
        "TRN2",
        target_bir_lowering=False,
        debug=False,
        enable_asserts=True,
        num_devices=NCORES,
    )
    x = nc.dram_tensor("x", [NSLOTS, P, 2, T, BLK], F16, kind="ExternalInput").ap()
    o = nc.dram_tensor("o", [NSLOTS, P, T, BLK], BF16, kind="ExternalOutput").ap()

    with tile.TileContext(nc) as tc, ExitStack() as ctx:
        const_pool = ctx.enter_context(tc.tile_pool(name="const", bufs=1))
        ident = const_pool.tile([P, P], F32)
        masks.make_identity(nc, ident[:])
        ones = const_pool.tile([P, 1], BF16)
        nc.gpsimd.memset(ones[:], 1.0)
        cbias = const_pool.tile([P, 1], F32)
        nc.gpsimd.memset(cbias[:], -C_BIAS)

        x_pool = ctx.enter_context(tc.tile_pool(name="x_sb", bufs=4))
        z_pool = ctx.enter_context(tc.tile_pool(name="z_sb", bufs=4))
        tmp_pool = ctx.enter_context(tc.tile_pool(name="tmp", bufs=3))
        o_pool = ctx.enter_context(tc.tile_pool(name="o_sb", bufs=3))
        s_pool = ctx.enter_context(tc.tile_pool(name="small", bufs=6))
        t_pool = ctx.enter_context(tc.tile_pool(name="tsb", bufs=4))
        ps_sb_pool = ctx.enter_context(tc.tile_pool(name="ps_sb", bufs=2, space="PSUM"))
        ps_t_pool = ctx.enter_context(tc.tile_pool(name="ps_t", bufs=2, space="PSUM"))
        ps_n_pool = ctx.enter_context(tc.tile_pool(name="ps_n", bufs=1, space="PSUM"))

        for k in range(NSLOTS):
            diag = k >= NSLOTS - NDIAG
            x_sb = x_pool.tile([P, 2, T, BLK], F16)
            nc.sync.dma_start(x_sb[:, 0], x[k][:, 0])
            nc.sync.dma_start(x_sb[:, 1], x[k][:, 1])

            za = z_pool.tile([P, T, BLK], BF16, name="za")
            zbt = z_pool.tile([P, T, BLK], BF16, name="zbt")
            nc.scalar.activation(za[:], x_sb[:, 0], AF.Exp, scale=1.0 / TAU,
                                 bias=cbias[:])
            nc.scalar.activation(zbt[:], x_sb[:, 1], AF.Exp, scale=1.0 / TAU,
                                 bias=cbias[:])

            # A-side group sums on DVE: sa[p, t*NG+g] over 256-wide col groups.
            # Diagonal slots skip this: B == A.T there, so the PE-computed
            # ps_sb below already holds A's row-group sums (columns permuted).
            if not diag:
                sa = s_pool.tile([P, NX], F32, name="sa")
                nc.vector.tensor_reduce(
                    sa[:],
                    za[:].rearrange("p t b -> p (t b)").rearrange(
                        "p (x s) -> p x s", s=GRP
                    ),
                    axis=mybir.AxisListType.X,
                    op=OP.add,
                )

            # B-side group sums on PE: zbt[p, t, y] = exp(B[y, t*128+p]) so a
            # ones-matmul contracts B-columns; accumulating subtile pairs
            # gives ps_sb[p, g*T+rc] = sum over group g for row rc*128+p.
            ps_sb = ps_sb_pool.tile([P, NX], F32)
            for g in range(NG):
                for rc in range(T):
                    for i in range(2):
                        nc.tensor.matmul(
                            ps_sb[:, g * T + rc:g * T + rc + 1],
                            zbt[:, 2 * g + i, rc * P:(rc + 1) * P],
                            ones[:],
                            start=(i == 0),
                            stop=(i == 1),
                        )

            # Reciprocals packed [ra | rb] (natural column order: ra x=t*NG+g,
            # rb x=8+g*T+rc), PE-transposed in fp32, cast to bf16 on the way
            # out of PSUM, then a single tiny SBUF->SBUF DMA flattens all 16
            # rows onto partition 0 so every matmul operand slice below is
            # base-partition-0 legal.
            r32 = s_pool.tile([P, 2, NX], F32, name="r32")
            if diag:
                nc.vector.reciprocal(
                    r32[:, 0].rearrange("p (t g) -> p t g", g=NG),
                    ps_sb[:].rearrange("p (g t) -> p t g", g=NG),
                )
            else:
                nc.vector.reciprocal(r32[:, 0], sa[:])
            nc.vector.reciprocal(r32[:, 1], ps_sb[:])
            ps_t = ps_t_pool.tile([2 * NX, P], F32)
            nc.tensor.transpose(
                ps_t[:], r32[:].rearrange("p a b -> p (a b)"), ident[:]
            )
            t_sb = t_pool.tile([2 * NX, P], BF16)
            nc.vector.tensor_scalar_mul(t_sb[:], ps_t[:], 1.0)
            rt = t_pool.tile([1, 2 * NX, P], BF16, name="rt")
            nc.gpsimd.dma_start(rt[:], t_sb[:])

            # tmp = za * zbt in bf16: row-subtiles 0-1 on the (otherwise idle)
            # gpsimd engine, 2-3 on DVE in 2x mode.
            tmp = tmp_pool.tile([P, T, BLK], BF16)
            nc.gpsimd.tensor_mul(tmp[:, 0:2], za[:, 0:2], zbt[:, 0:2])
            nc.vector.tensor_mul(tmp[:, 2:4], za[:, 2:4], zbt[:, 2:4])

            # psum_norm[v][p, f] = ra[p, (v, f//256)] * rb[f, v//2] as K=1
            # outer-product matmuls (one per (v, f-half)); then
            # out[v] = tmp[v] * psum_norm[v] per DVE op.
            o_sb = o_pool.tile([P, T, BLK], BF16)
            for hv in range(2):
                ps_n = ps_n_pool.tile([P, 2, BLK], F32, name=f"psn{hv}")
                for dv in range(2):
                    v = 2 * hv + dv
                    g = v // 2
                    for h in range(2):
                        nc.tensor.matmul(
                            ps_n[:, dv, h * GRP:(h + 1) * GRP],
                            rt[0:1, v * NG + h, :],
                            rt[0:1, NX + g * T + 2 * h:NX + g * T + 2 * h + 2, :]
                            .rearrange("o a p -> o (a p)"),
                        )
                if hv == 0:
                    # ACT (idle after its exps) drains this half of PSUM to
                    # SBUF bf16 so the combine below runs in DVE 2x mode.
                    n_sb = s_pool.tile([P, 2, BLK], BF16, name="n_sb")
                    nc.scalar.copy(n_sb[:], ps_n[:])
                    nc.vector.tensor_mul(
                        o_sb[:, 0:2, :], tmp[:, 0:2, :], n_sb[:]
                    )
                else:
                    nc.vector.tensor_mul(
                        o_sb[:, 2:4, :], tmp[:, 2:4, :], ps_n[:]
                    )

            nc.gpsimd.dma_start(o[k], o_sb[:])

    nc.compile()
    return nc


_NC = None


def _get_nc():
    global _NC
    if _NC is None:
        _NC = build()
    return _NC


def _to_pmajor(block: np.ndarray) -> np.ndarray:
    # (512, 512) row-major -> (128, 4, 512): row r = t*P + p lands at [p, t, :]
    return block.reshape(T, P, BLK).transpose(1, 0, 2)


def make_in_maps(sims: np.ndarray) -> list[dict[str, np.ndarray]]:
    s16 = sims.astype(np.float16)
    in_maps = []
    for c in range(NCORES):
        xs = np.empty((NSLOTS, P, 2, T, BLK), np.float16)
        for k, (i, j) in enumerate(CORE_PAIRS[c]):
            a = s16[i * BLK:(i + 1) * BLK, j * BLK:(j + 1) * BLK]
            bt = s16[j * BLK:(j + 1) * BLK, i * BLK:(i + 1) * BLK].T
            if i == j:
                a = a.copy()
                np.fill_diagonal(a, MASK)
                bt = a.T
            xs[k, :, 0] = _to_pmajor(np.ascontiguousarray(a))
            xs[k, :, 1] = _to_pmajor(np.ascontiguousarray(bt))
        in_maps.append({"x": xs})
    return in_maps


def assemble(results: list[dict[str, np.ndarray]]) -> np.ndarray:
    out = np.empty((N, N), np.float32)
    for c in range(NCORES):
        o_pm = np.asarray(results[c]["o"]).astype(np.float32)
        o_stack = o_pm.transpose(0, 2, 1, 3).reshape(NSLOTS, BLK, BLK)
        for k, (i, j) in enumerate(CORE_PAIRS[c]):
            out[i * BLK:(i + 1) * BLK, j * BLK:(j + 1) * BLK] = o_stack[k]
            if i != j:
                out[j * BLK:(j + 1) * BLK, i * BLK:(i + 1) * BLK] = o_stack[k].T
    return out


def run_on_hw(sims: np.ndarray, **spmd_kwargs):
    """Run the kernel on the 8 NeuronCores. Returns (out, BassKernelResults).

    The device occasionally throws a transient NRT_EXEC_UNIT_UNRECOVERABLE
    and needs ~a minute to come back, so failed runs are retried."""
    import time

    nc = _get_nc()
    in_maps = make_in_maps(sims)
    last_exc = None
    for attempt in range(3):
        if attempt:
            time.sleep(75)
        try:
            res = run_bass_kernel_spmd(
                nc, in_maps, core_ids=list(range(NCORES)), **spmd_kwargs
            )
            return assemble(res.results), res
        except Exception as exc:  # noqa: BLE001 - device flake, retry
            last_exc = exc
    raise last_exc


def kernel(similarities: np.ndarray) -> np.ndarray:
    sims = np.ascontiguousarray(similarities, dtype=np.float32)
    assert sims.shape == (N, N)
    out, _ = run_on_hw(sims)
    return out


if __name__ == "__main__":
    rng = np.random.default_rng(0)
    sims = rng.standard_normal((N, N), dtype=np.float32)
    out = kernel(similarities=sims)
    print("out", out.shape, out.dtype, float(out.max()))
